# revision 41
# baseline (speedup 1.0000x reference)
"""Multi-head attention (B=2, S=2048, D=1024, H=16) on 8 Trainium2 cores.

Sharding: head-group parallel (2 heads per core) for QKV projections +
attention; two 8-rank AllToAlls (one per batch) redistribute context from
head-sharding to token-sharding; each core then runs the output projection
(full W_o) for its 2x256-token chunk (256 tokens of each batch, offset
core*256 within the batch). Splitting the collective per batch lets the
first A2A and half the output projection overlap batch-1 attention.

All matmuls bf16 with fp32 PSUM accumulation; softmax in fp32 (no max
subtraction -- scores are bounded ~|2.5|); denominators via an appended
ones-column in the AV stationary operand; normalization via DVE reciprocal
+ gpsimd partition-broadcast (PE stays out of the chain).

Per-core layouts (features on partitions, "transposed"):
  xt  [D=1024, T=4096]  bf16  X^T, replicated
  wq/wk/wv [128, 8*128] bf16  core's 2-head column slice, SBUF layout
                              (partition p holds rows {d*128+p}, d-major)
  wo  [128, 8*1024]     bf16  replicated, same SBUF layout
  bo  [128, 8]          f32   replicated, partition-major
  msk [128, 1408]       bf16  packed causal masks for diagonal k-tile
                              offsets 0..3 (widths 512/512/256/128; tiles
                              at offsets 2..3 skip their fully-masked
                              column prefix everywhere)
  out [D, 512]          f32   out^T: cols 0:256 batch-0 tokens
                              [c*256,(c+1)*256), cols 256:512 same of batch 1
"""

import os
import sys
from contextlib import ExitStack

for _p in ("/opt/trn_rl_repo",):
    if os.path.isdir(_p) and _p not in sys.path:
        sys.path.insert(0, _p)

import numpy as np
import ml_dtypes

import concourse.bass as bass
import concourse.tile as tile
from concourse import bacc, mybir
from concourse.bass import ts
from concourse.bass_utils import run_bass_kernel_spmd

BF16 = ml_dtypes.bfloat16
BF = mybir.dt.bfloat16
F32 = mybir.dt.float32

B, S, D, H, DH = 2, 2048, 1024, 16, 64
NCORES = 8
T = B * S              # 4096 flattened tokens
FPC = D // NCORES      # 128 features per core (2 heads)
CH2 = S // NCORES      # 256 tokens per (core, batch) in the output phase
DT = D // 128          # 8 contraction tiles over D
QT = 512               # attention q-tile
NQ = S // QT           # 4 q-tiles per (batch, head)
NKT = S // 128         # 16 k-tiles per (batch, head)
UNITS = B * (FPC // DH)  # 4 attention units per core: (batch, local head)
GRP = 2                # k-tiles per PSUM score group (2 banks)
MW = [QT, QT, QT - 256, QT - 384]   # packed causal-mask block widths, o=0..3
MOFF = [0, QT, 2 * QT, 3 * QT - 256]  # block offsets
MTOT = 3 * QT - 128

_BUILD_CACHE = {}


def _build(amp=1, collective=True, num_devices=NCORES, compile=True,
           phases="qkv,attn,proj"):
    key = (amp, collective, num_devices, compile, phases)
    if key in _BUILD_CACHE:
        return _BUILD_CACHE[key]
    nc = bacc.Bacc("TRN2", target_bir_lowering=False, debug=False,
                   num_devices=num_devices)
    xt = nc.dram_tensor("xt", [D, T], BF, kind="ExternalInput").ap()
    wq = nc.dram_tensor("wq", [128, DT * FPC], BF, kind="ExternalInput").ap()
    wk = nc.dram_tensor("wk", [128, DT * FPC], BF, kind="ExternalInput").ap()
    wv = nc.dram_tensor("wv", [128, DT * FPC], BF, kind="ExternalInput").ap()
    wo = nc.dram_tensor("wo", [128, DT * D], BF, kind="ExternalInput").ap()
    bo = nc.dram_tensor("bo", [128, DT], F32, kind="ExternalInput").ap()
    msk = nc.dram_tensor("msk", [128, MTOT], BF, kind="ExternalInput").ap()
    out = nc.dram_tensor("out", [D, 2 * CH2], F32, kind="ExternalOutput").ap()

    with tile.TileContext(nc) as tc, ExitStack() as ctx:
        pers = ctx.enter_context(tc.tile_pool(name="pers", bufs=1))
        # PSUM: tag "big" = 3 slots x 2 banks (scores groups, QKV/outproj
        # outputs) + tag "pctx" = 2 slots x 1 bank (AV accumulator rows
        # 0..64) = 8 banks.
        ps = ctx.enter_context(tc.tile_pool(name="ps", bufs=2, space="PSUM"))
        work = ctx.enter_context(tc.tile_pool(name="work", bufs=6))
        sm = ctx.enter_context(tc.tile_pool(name="sm", bufs=8))
        dram = ctx.enter_context(tc.tile_pool(name="dram", bufs=1, space="DRAM"))

        # ---- persistent SBUF tensors
        xts = pers.tile([128, DT * T], BF, tag="xts")
        xts_d = [xts[:, d * T:(d + 1) * T] for d in range(DT)]
        qts = pers.tile([128, T], BF, tag="qts")
        kts = pers.tile([128, T], BF, tag="kts")
        # Vn per (batch, k-tile): [128 tokens, 130] = [V_h0 | 1 | V_h1 | 1]
        vns = pers.tile([128, B * NKT * 130], BF, tag="vns")
        wqs = pers.tile([128, DT * FPC], BF, tag="wqs")
        wks = pers.tile([128, DT * FPC], BF, tag="wks")
        wvs = pers.tile([128, DT * FPC], BF, tag="wvs")
        wos = pers.tile([128, DT * D], BF, tag="wos")
        mks = pers.tile([128, MTOT], BF, tag="mks")
        bos = pers.tile([128, DT], F32, tag="bos")
        a2s = [pers.tile([128, NCORES * CH2], BF, tag=f"a2s{b}",
                         name=f"a2s{b}") for b in range(B)]
        ots = [pers.tile([128, DT * CH2], F32, tag=f"ots{b}",
                         name=f"ots{b}") for b in range(B)]

        def vn_ap(u, t):
            b, hl = u // (FPC // DH), u % (FPC // DH)
            o = (b * NKT + t) * 130 + hl * 65
            return vns[:, o:o + 65]

        # ---- load weights / constants; weights are host-prepped to the
        # SBUF layout (one DMA each); X^T goes chunk-major (one 1MB DMA per
        # 512-token chunk covering all d-tiles) so projections start early.
        xts_3d = xts[:].rearrange("p (d t) -> p d t", t=T)
        xt_3d = xt.rearrange("(d p) t -> p d t", p=128)
        nc.sync.dma_start(wqs[:], wq[:])
        nc.sync.dma_start(xts_3d[:, :, ts(0, T // 16)],
                          xt_3d[:, :, ts(0, T // 16)])
        nc.sync.dma_start(wks[:], wk[:])
        nc.sync.dma_start(xts_3d[:, :, ts(1, T // 16)],
                          xt_3d[:, :, ts(1, T // 16)])
        nc.sync.dma_start(wvs[:], wv[:])
        for c8 in range(1, 8):
            nc.sync.dma_start(xts_3d[:, :, ts(c8, T // 8)],
                              xt_3d[:, :, ts(c8, T // 8)])
        nc.sync.dma_start(mks[:], msk[:])
        nc.sync.dma_start(bos[:], bo[:])
        nc.sync.dma_start(wos[:], wo[:])
        nc.vector.memset(
            vns[:].rearrange("p (n c) -> p n c", c=65)[:, :, 64:65], 1.0)

        a2a_in = [dram.tile([NCORES, FPC, CH2], BF, tag=f"a2a_in{b}",
                          name=f"a2a_in{b}") for b in range(B)]
        a2a_out = [dram.tile([NCORES, FPC, CH2], BF, tag=f"a2a_out{b}",
                           name=f"a2a_out{b}") for b in range(B)]

        def qk_chunk(wsb, dst, off, w):
            pt_full = ps.tile([128, GRP * QT], F32, tag="big", bufs=3)
            pt = pt_full[:, 0:w]
            for d in range(DT):
                nc.tensor.matmul(
                    pt[:], wsb[:, ts(d, FPC)], xts_d[d][:, off:off + w],
                    start=(d == 0), stop=(d == DT - 1))
            nc.vector.tensor_copy(dst[:, off:off + w], pt[:])

        def v_tile(b, tt):
            # V natural, both heads: one [128 tokens, 128 feats] tile
            tg = b * (S // 128) + tt
            pvt_full = ps.tile([128, GRP * QT], F32, tag="big", bufs=3)
            pvt = pvt_full[:, 0:FPC]
            for d in range(DT):
                nc.tensor.matmul(
                    pvt[:], xts_d[d][:, ts(tg, 128)], wvs[:, ts(d, FPC)],
                    start=(d == 0), stop=(d == DT - 1))
            # both heads in one strided copy: cols {0..63, 65..128}
            o = (b * NKT + tt) * 130
            nc.vector.tensor_copy(
                vns[:, o:o + 130].rearrange(
                    "p (h c) -> p h c", c=65)[:, :, 0:DH],
                pvt[:, 0:FPC].rearrange("p (h c) -> p h c", c=DH))

        def qkv_proj(b):
            # chunk-major: Q, K, V consumed per 512-token chunk so each
            # arriving X^T chunk feeds ~5us of PE work (DMA needs ~3us)
            for ch in range(S // 512):
                off = b * S + ch * 512
                if b == 0 and ch == 0:
                    for o0 in (0, 256):
                        qk_chunk(wqs, qts, off + o0, 256)
                    qk_chunk(wks, kts, off, 512)
                else:
                    qk_chunk(wqs, qts, off, 512)
                    qk_chunk(wks, kts, off, 512)
                for tt in range(4 * ch, 4 * ch + 4):
                    v_tile(b, tt)

        def attention(u):
            b, hl = u // (FPC // DH), u % (FPC // DH)
            qoff = b * S
            frow = hl * DH
            for j in range(NQ):
                nkt = (j + 1) * (QT // 128)
                # Diagonal pairs first ([o=0,1] then [o=2,3]) so masks
                # resolve early and the first AV matmul (o=0, full width)
                # initializes the whole PSUM accumulator region; diagonal
                # tiles at offsets 2..3 skip their fully-masked column
                # prefix (packed into sct/ext).
                d0 = (j + 1) * 4 - 4
                groups = [[d0, d0 + 1], [d0 + 2, d0 + 3]] + \
                    [[t, t + 1] for t in range(d0 - 2, -1, -2)]
                cpt = ps.tile([128, QT], F32, tag="pctx")
                n_av = 0
                for grp in groups:
                    sct = ps.tile([128, GRP * QT], F32, tag="big", bufs=3)
                    ext = work.tile([128, GRP * QT], BF, tag="exp")
                    segs = []  # (tile, valid_start, packed_offset)
                    po = 0
                    for t in grp:
                        o = t - j * (QT // 128)
                        vs = o * 128 if o >= 2 else 0
                        segs.append((t, o, vs, po))
                        po += QT - vs
                    for t, o, vs, p in segs:
                        nc.tensor.matmul(
                            sct[:, p:p + QT - vs],
                            kts[frow:frow + DH,
                                qoff + t * 128: qoff + t * 128 + 128],
                            qts[frow:frow + DH,
                                qoff + j * QT + vs: qoff + (j + 1) * QT],
                            start=True, stop=True)
                    nc.scalar.activation(
                        ext[:, 0:po], sct[:, 0:po],
                        mybir.ActivationFunctionType.Exp,
                        scale=float(1.0 / np.sqrt(DH)))
                    # masks: diagonal tiles are contiguous trailing segs;
                    # packed msk blocks at MOFF[o] with width QT-vs
                    dsegs = [s for s in segs if s[1] >= 0]
                    if dsegs:
                        p0 = dsegs[0][3]
                        w = po - p0
                        nc.vector.tensor_mul(
                            ext[:, p0:p0 + w], ext[:, p0:p0 + w],
                            mks[:, MOFF[dsegs[0][1]]:MOFF[dsegs[0][1]] + w])
                    for t, o, vs, p in segs:
                        nc.tensor.matmul(
                            cpt[0:65, vs:QT], vn_ap(u, t),
                            ext[:, p:p + QT - vs],
                            start=(n_av == 0), stop=(n_av == nkt - 1))
                        n_av += 1
                # normalize: denom row 64 -> recip -> gpsimd broadcast across
                # partitions (keeps PE out of the chain) -> multiply
                rc = sm.tile([1, QT], F32, tag="rc")
                nc.vector.reciprocal(rc[:], cpt[64:65, :])
                rcb = sm.tile([64, QT], F32, tag="rcb")
                nc.gpsimd.partition_broadcast(rcb[:], rc[:], channels=64)
                ctt = sm.tile([64, QT], BF, tag="ctt")
                nc.vector.tensor_mul(ctt[:], cpt[0:64, :], rcb[:])
                # q-tile j covers peers 2j (cols 0:256) and 2j+1 (256:512);
                # one DMA, partition-first on both sides
                nc.sync.dma_start(
                    a2a_in[b][2 * j:2 * j + 2, frow:frow + DH, :]
                    .rearrange("g p c -> p g c"),
                    ctt[:].rearrange("p (g c) -> p g c", g=2))

        def a2a(b):
            if collective:
                nc.gpsimd.collective_compute(
                    "AllToAll", mybir.AluOpType.bypass,
                    replica_groups=[list(range(NCORES))],
                    ins=[a2a_in[b].opt()], outs=[a2a_out[b].opt()])
            else:
                nc.sync.dma_start(a2a_out[b][:], a2a_in[b][:])
            # a2s load on the SP queue right behind the collective: only
            # later ctt stores queue after it there, and those aren't
            # needed until the next collective anyway. Two halves so the
            # out-proj's first peer-matmuls start after half the load.
            for hj in range(2):
                nc.sync.dma_start(
                    a2s[b][:, hj * 4 * CH2:(hj + 1) * 4 * CH2].rearrange(
                        "p (j c) -> p j c", c=CH2),
                    a2a_out[b][hj * 4:(hj + 1) * 4].rearrange(
                        "j p c -> p j c"))

        def out_proj(b, store=True):
            # output projection for this core's batch-b 256-token chunk;
            # store=False re-runs are idempotent PE filler that keeps the
            # array warm across the batch-1 redistribute latency
            for f in range(DT):
                pot_full = ps.tile([128, GRP * QT], F32, tag="big", bufs=3)
                pot = pot_full[:, 0:CH2]
                for jb in range(NCORES):
                    nc.tensor.matmul(
                        pot[:], wos[:, jb * D + f * 128: jb * D + (f + 1) * 128],
                        a2s[b][:, ts(jb, CH2)],
                        start=(jb == 0), stop=(jb == NCORES - 1))
                if store:
                    nc.vector.tensor_scalar_add(
                        ots[b][:, ts(f, CH2)], pot[:], bos[:, f:f + 1])
                if store and f % 2 == 1:
                    # store per f-pair so stores overlap later matmuls
                    h = f // 2
                    nc.scalar.dma_start(
                        out.rearrange("(g f p) c -> p g f c",
                                      p=128, g=4)[:, h, :, ts(b, CH2)],
                        ots[b][:].rearrange("p (g f c) -> p g f c",
                                            c=CH2, g=4)[:, h])

        for _rep in range(amp):
            # batch-0 QKV + attention, then its A2A overlaps batch-1 work;
            # batch-0 out-proj runs under attn(3)'s softmax tail + a2a(1).
            qkv_proj(0)
            attention(0)
            attention(1)
            a2a(0)
            qkv_proj(1)
            attention(2)
            attention(3)
            a2a(1)
            out_proj(0)
            out_proj(0, store=False)
            out_proj(1)

        if os.environ.get("BASS_MHA_DEBUG", "0") == "1":
            dbg = {"dbg_q": qts, "dbg_k": kts, "dbg_v": vns,
                   "dbg_a2s0": a2s[0], "dbg_a2s1": a2s[1]}
            for nm, t in dbg.items():
                dt_ = nc.dram_tensor(nm, list(t.shape), t.dtype,
                                     kind="ExternalOutput").ap()
                nc.sync.dma_start(dt_, t[:])
            for b in range(B):
                dt_ = nc.dram_tensor(f"dbg_ain{b}", [NCORES, FPC, CH2],
                                     BF, kind="ExternalOutput").ap()
                nc.sync.dma_start(dt_, a2a_in[b][:])

    if compile:
        nc.compile()
    _BUILD_CACHE[key] = nc
    return nc


def _sbuf_layout(w):
    # [D, F] -> [128, DT*F]: partition p holds rows {d*128+p}, d-major cols
    d, f = w.shape
    return np.ascontiguousarray(
        w.reshape(d // 128, 128, f).transpose(1, 0, 2).reshape(128, -1))


def _make_inputs(X, W_q, W_k, W_v, W_o, b_o):
    Xf = np.asarray(X, np.float32).reshape(T, D)
    xt = np.ascontiguousarray(Xf.T).astype(BF16)
    wo = _sbuf_layout(np.asarray(W_o, np.float32)).astype(BF16)
    bo = np.ascontiguousarray(
        np.asarray(b_o, np.float32).reshape(DT, 128).T)
    kk = np.arange(128)[:, None]
    msk = np.concatenate(
        [(np.arange(MW[o])[None, :] >= kk + (128 if o == 1 else 0))
         for o in range(4)], axis=1).astype(BF16)
    in_maps = []
    for c in range(NCORES):
        sl = slice(c * FPC, (c + 1) * FPC)
        in_maps.append({
            "xt": xt,
            "wq": _sbuf_layout(np.asarray(W_q, np.float32)[:, sl]).astype(BF16),
            "wk": _sbuf_layout(np.asarray(W_k, np.float32)[:, sl]).astype(BF16),
            "wv": _sbuf_layout(np.asarray(W_v, np.float32)[:, sl]).astype(BF16),
            "wo": wo,
            "bo": bo,
            "msk": msk,
        })
    return in_maps


def kernel(X, W_q, W_k, W_v, W_o, b_o):
    nc = _build()
    in_maps = _make_inputs(X, W_q, W_k, W_v, W_o, b_o)
    res = run_bass_kernel_spmd(nc, in_maps, list(range(NCORES)))
    out_t = np.empty((D, T), np.float32)
    for c in range(NCORES):
        o = res.results[c]["out"]  # [D, 512]
        out_t[:, c * CH2:(c + 1) * CH2] = o[:, 0:CH2]
        out_t[:, S + c * CH2:S + (c + 1) * CH2] = o[:, CH2:2 * CH2]
    return np.ascontiguousarray(out_t.T).reshape(B, S, D).astype(np.float32)
